# revision 5
# baseline (speedup 1.0000x reference)
"""Trainium2 Bass kernel for nn_Attention_55130200211640.

GQA attention block: q/k/v projections + RoPE (theta=1e6) + non-causal
softmax attention (16 q-heads, 4 kv-heads, head_dim 128) + output projection.
B=4, S=2048, HID=2048, fp32 I/O.

Sharding: data-parallel over (batch x 4, query-half x 2) = 8 cores, no
collectives. Each core computes K/V for the full sequence of its batch
(k/v projection duplicated across the pair of cores sharing a batch) and
attention + o_proj for its 1024 queries.

All matmuls run in float32r (TF32-like reduced-precision fp32 path on the
PE: ~1 cycle/row at N>=512 vs 4 for plain fp32, rel. err ~1e-4).

Device dataflow (per core), everything in transposed "contraction-on-
partition" layouts:
  phase 1: X^T resident in SBUF; compute Q^T [hd,i], K^T [d_kv,j] (both
           RoPE'd on DVE using host-transposed cos/sin tables) and
           V [j,d_kv]; bounce all three through DRAM scratch (SBUF is too
           small to hold X^T and the QKV results simultaneously).
  phase 2: per q-head h: S^T[j,i] = K^T_g . Q^T_h on PE; E = exp(S^T/sqrt(D))
           on ACT (scale folded into the activation); U^T[d,i] = sum_j V E
           and Z[i] = sum_j E (ones-matmul) accumulated on PE;
           O^T_h = U^T * recip(Z) on DVE (deferred softmax normalization).
  phase 3: y[i,o] = sum_h O^T_h . wo^T_h accumulated in PSUM over heads.

Token columns are host-permuted so each core's 1024 queries are columns
0:1024 of its X^T — softmax/attention are permutation-invariant over keys,
so K/V in permuted order is fine as long as cos/sin tables are permuted
consistently (they are, per core).
"""

import numpy as np

B, S, HID = 4, 2048, 2048
H, KV, D = 16, 4, 128
REP = H // KV
N_CORES = 8
QLEN = S // 2          # queries per core
CT = HID // 128        # contraction tiles
SCALE = 1.0 / float(np.sqrt(D))

_cache = {}


def _emit(nc, tc, io):
    import concourse.mybir as mybir
    import concourse.tile as tile

    F32 = mybir.dt.float32
    F32R = mybir.dt.float32r
    Exp = mybir.ActivationFunctionType.Exp

    xt_d, cosk_d, sinkm_d, wqt_d, wkt_d, wvt_d, wot_d, ones_d, y_d = io

    from contextlib import ExitStack
    ctx = ExitStack()

    # ---- DRAM scratch for the QKV bounce ----
    dram = ctx.enter_context(tc.tile_pool(name="dram", bufs=1, space="DRAM"))
    qt_s = dram.tile([H, 128, QLEN], F32R)      # Q^T per head [d, i]
    kt_s = dram.tile([KV, 128, S], F32R)        # K^T per kv-head [d, j]
    v_s = dram.tile([S // 128, 128, KV * D], F32R)  # V [j, d_kv]

    # ---- persistent PSUM pool: 4 slots x [128,1024]f32 = 8 banks ----
    ps_pool = ctx.enter_context(tc.tile_pool(name="ps", bufs=1, space="PSUM"))

    PSUM_BUFS = {"A": 2, "B": 1, "C": 1}

    def psum(tag, shape=(128, 1024)):
        return ps_pool.tile(list(shape), F32, name=f"ps_{tag}", tag=tag,
                            bufs=PSUM_BUFS[tag])

    const_pool = ctx.enter_context(tc.tile_pool(name="const", bufs=1))
    ones_t = const_pool.tile([128, 128], F32R)
    nc.sync.dma_start(ones_t[:], ones_d[:])

    # ================= phase 1: projections =================
    with (
        tc.tile_pool(name="p1x", bufs=1) as p1x,
        tc.tile_pool(name="p1cs", bufs=1) as p1cs,
        tc.tile_pool(name="p1w", bufs=2) as p1w,
        tc.tile_pool(name="p1st", bufs=2) as p1st,
    ):
        XT = p1x.tile([128, CT, S], F32R)
        nc.sync.dma_start(XT[:], xt_d.rearrange("(ct p) j -> p ct j", p=128))
        COS = p1cs.tile([128, S], F32)
        nc.sync.dma_start(COS[:], cosk_d[:])
        SINM = p1cs.tile([128, S], F32)
        nc.sync.dma_start(SINM[:], sinkm_d[:])

        def rope_store(ps, cs_lo, cs_hi, dst):
            """RoPE a [128,512] psum tile (layout [d, pos]) -> f32r -> DMA out.

            cs_lo:cs_hi is the column slice into COS/SINM for these positions.
            """
            tmp = p1st.tile([128, 512], F32R, tag="tmp")
            stage = p1st.tile([128, 512], F32R, tag="stage")
            nc.vector.tensor_mul(tmp[:], ps[:], COS[:, cs_lo:cs_hi])
            nc.vector.tensor_mul(stage[0:64, :], ps[64:128, :], SINM[0:64, cs_lo:cs_hi])
            nc.vector.tensor_mul(stage[64:128, :], ps[0:64, :], SINM[64:128, cs_lo:cs_hi])
            nc.vector.tensor_add(stage[:], stage[:], tmp[:])
            nc.sync.dma_start(dst, stage[:])

        # --- Q^T: 16 head tiles x 2 i-blocks of 512 ---
        wqt_r = wqt_d.rearrange("(ct p) m -> p ct m", p=128)
        for h in range(H):
            wt = p1w.tile([128, CT, 128], F32R, tag="w", bufs=2)
            nc.sync.dma_start(wt[:], wqt_r[:, :, h * 128:(h + 1) * 128])
            for ib in range(2):
                i0 = ib * 512
                ps = psum("A")
                for ct in range(CT):
                    nc.tensor.matmul(
                        ps[:, 0:512], wt[:, ct, :], XT[:, ct, i0:i0 + 512],
                        start=(ct == 0), stop=(ct == CT - 1),
                    )
                rope_store(ps[:, 0:512], i0, i0 + 512, qt_s[h, :, i0:i0 + 512])

        # --- K^T: 4 kv-head tiles x 4 j-blocks of 512 ---
        wkt_r = wkt_d.rearrange("(ct p) m -> p ct m", p=128)
        for g in range(KV):
            wt = p1w.tile([128, CT, 128], F32R, tag="w", bufs=2)
            nc.sync.dma_start(wt[:], wkt_r[:, :, g * 128:(g + 1) * 128])
            for jb in range(4):
                j0 = jb * 512
                ps = psum("B" if jb % 2 == 0 else "C")
                for ct in range(CT):
                    nc.tensor.matmul(
                        ps[:, 0:512], wt[:, ct, :], XT[:, ct, j0:j0 + 512],
                        start=(ct == 0), stop=(ct == CT - 1),
                    )
                rope_store(ps[:, 0:512], j0, j0 + 512, kt_s[g, :, j0:j0 + 512])

        # --- V: 16 j-tiles x 2 halves of the 512 kv-dims ---
        wvt_r = wvt_d.rearrange("(ct p) m -> p ct m", p=128)
        for vh in range(2):
            d0 = vh * 256
            wt = p1w.tile([128, CT, 256], F32R, tag="w", bufs=2)
            nc.sync.dma_start(wt[:], wvt_r[:, :, d0:d0 + 256])
            for jt in range(S // 128):
                ps = psum("A" if jt % 2 == 0 else "B")
                for ct in range(CT):
                    nc.tensor.matmul(
                        ps[:, 0:256], XT[:, ct, jt * 128:(jt + 1) * 128],
                        wt[:, ct, :],
                        start=(ct == 0), stop=(ct == CT - 1),
                    )
                stage = p1st.tile([128, 256], F32R, tag="vstage", bufs=3)
                nc.scalar.copy(stage[:], ps[:, 0:256])
                nc.sync.dma_start(v_s[jt, :, d0:d0 + 256], stage[:])

    # ================= phase 2: attention =================
    o_pool = ctx.enter_context(tc.tile_pool(name="ot", bufs=1))
    OT = o_pool.tile([128, H, QLEN], F32R)  # O^T accum, 8 MiB

    with (
        tc.tile_pool(name="p2kv", bufs=1) as p2kv,
        tc.tile_pool(name="p2q", bufs=3) as p2q,
        tc.tile_pool(name="p2e", bufs=6) as p2e,
        tc.tile_pool(name="p2r", bufs=2) as p2r,
    ):
        KT = p2kv.tile([128, KV, S], F32R)
        for g in range(KV):
            nc.sync.dma_start(KT[:, g, :], kt_s[g, :, :])
        VV = p2kv.tile([128, S // 128, KV * D], F32R)
        for jt in range(S // 128):
            nc.sync.dma_start(VV[:, jt, :], v_s[jt, :, :])

        for h in range(H):
            g = h // REP
            QH = p2q.tile([128, QLEN], F32R, tag="qh", bufs=3)
            nc.sync.dma_start(QH[:], qt_s[h, :, :])
            U_ps = psum("B")
            Z_ps = psum("C")
            for jt in range(S // 128):
                S_ps = psum("A")
                kt_sl = KT[:, g, jt * 128:(jt + 1) * 128]
                nc.tensor.matmul(S_ps[:, 0:512], kt_sl, QH[:, 0:512],
                                 start=True, stop=True)
                nc.tensor.matmul(S_ps[:, 512:1024], kt_sl, QH[:, 512:1024],
                                 start=True, stop=True)
                E = p2e.tile([128, QLEN], F32R, tag="e", bufs=6)
                nc.scalar.activation(E[:], S_ps[:], Exp, scale=SCALE)
                v_sl = VV[:, jt, g * 128:(g + 1) * 128]
                st, sp = (jt == 0), (jt == S // 128 - 1)
                nc.tensor.matmul(U_ps[:, 0:512], v_sl, E[:, 0:512],
                                 start=st, stop=sp)
                nc.tensor.matmul(U_ps[:, 512:1024], v_sl, E[:, 512:1024],
                                 start=st, stop=sp)
                nc.tensor.matmul(Z_ps[:, 0:512], ones_t[:], E[:, 0:512],
                                 start=st, stop=sp)
                nc.tensor.matmul(Z_ps[:, 512:1024], ones_t[:], E[:, 512:1024],
                                 start=st, stop=sp)
            RZ = p2r.tile([128, QLEN], F32, tag="rz", bufs=2)
            nc.vector.reciprocal(RZ[:], Z_ps[:])
            nc.vector.tensor_mul(OT[:, h, :], U_ps[:], RZ[:])

    # ================= phase 3: output projection =================
    with (
        tc.tile_pool(name="p3w", bufs=4) as p3w,
        tc.tile_pool(name="p3y", bufs=4) as p3y,
    ):
        for ob in range(4):
            o0 = ob * 512
            for i4 in range(2):
                pss = [psum("A"), psum("A"), psum("B"), psum("C")]
                for h in range(H):
                    W = p3w.tile([128, 512], F32R, tag="wo", bufs=4)
                    nc.sync.dma_start(W[:], wot_d[h * 128:(h + 1) * 128, o0:o0 + 512])
                    for q in range(4):
                        it = i4 * 4 + q
                        nc.tensor.matmul(
                            pss[q][:, 0:512],
                            OT[:, h, it * 128:(it + 1) * 128], W[:],
                            start=(h == 0), stop=(h == H - 1),
                        )
                for q in range(4):
                    it = i4 * 4 + q
                    yt = p3y.tile([128, 512], F32, tag="y", bufs=4)
                    nc.vector.tensor_copy(yt[:], pss[q][:, 0:512])
                    nc.sync.dma_start(
                        y_d[it * 128:(it + 1) * 128, o0:o0 + 512], yt[:])

    ctx.close()


def _build():
    import concourse.mybir as mybir
    import concourse.tile as tile
    from concourse import bacc

    F32 = mybir.dt.float32
    F32R = mybir.dt.float32r

    nc = bacc.Bacc("TRN2", target_bir_lowering=False, debug=False)
    xt_d = nc.dram_tensor("xt", [HID, S], F32R, kind="ExternalInput").ap()
    cosk_d = nc.dram_tensor("cosk", [128, S], F32, kind="ExternalInput").ap()
    sinkm_d = nc.dram_tensor("sinkm", [128, S], F32, kind="ExternalInput").ap()
    wqt_d = nc.dram_tensor("wqt", [HID, H * D], F32R, kind="ExternalInput").ap()
    wkt_d = nc.dram_tensor("wkt", [HID, KV * D], F32R, kind="ExternalInput").ap()
    wvt_d = nc.dram_tensor("wvt", [HID, KV * D], F32R, kind="ExternalInput").ap()
    wot_d = nc.dram_tensor("wot", [H * D, HID], F32R, kind="ExternalInput").ap()
    ones_d = nc.dram_tensor("ones", [128, 128], F32R, kind="ExternalInput").ap()
    y_d = nc.dram_tensor("y", [QLEN, HID], F32, kind="ExternalOutput").ap()

    with tile.TileContext(nc) as tc:
        _emit(nc, tc, (xt_d, cosk_d, sinkm_d, wqt_d, wkt_d, wvt_d, wot_d,
                       ones_d, y_d))
    nc.compile()
    return nc


class _Runner:
    """Persistent-jit PJRT executor (axon) / NRT executor (native)."""

    def __init__(self, nc):
        self.nc = nc
        from concourse._compat import axon_active
        self.axon = axon_active()
        if not self.axon:
            return
        import jax
        from jax.sharding import Mesh, PartitionSpec
        from jax.experimental.shard_map import shard_map
        import concourse.mybir as mybir
        from concourse.bass2jax import (
            _bass_exec_p, install_neuronx_cc_hook, partition_id_tensor)

        install_neuronx_cc_hook()
        partition_name = (nc.partition_id_tensor.name
                          if nc.partition_id_tensor else None)
        in_names, out_names, out_avals, zero_outs = [], [], [], []
        for alloc in nc.m.functions[0].allocations:
            if not isinstance(alloc, mybir.MemoryLocationSet):
                continue
            name = alloc.memorylocations[0].name
            if alloc.kind == "ExternalInput":
                if name != partition_name:
                    in_names.append(name)
            elif alloc.kind == "ExternalOutput":
                shape = tuple(alloc.tensor_shape)
                dtype = mybir.dt.np(alloc.dtype)
                out_names.append(name)
                out_avals.append(jax.core.ShapedArray(shape, dtype))
                zero_outs.append(np.zeros(shape, dtype))
        self.in_names, self.out_names = in_names, out_names
        self.zero_outs = zero_outs
        n_params, n_outs = len(in_names), len(out_names)
        all_in = in_names + out_names
        if partition_name is not None:
            all_in.append(partition_name)
        donate = tuple(range(n_params, n_params + n_outs))

        def _body(*args):
            operands = list(args)
            if partition_name is not None:
                operands.append(partition_id_tensor())
            return tuple(_bass_exec_p.bind(
                *operands,
                out_avals=tuple(out_avals),
                in_names=tuple(all_in),
                out_names=tuple(out_names),
                lowering_input_output_aliases=(),
                sim_require_finite=True,
                sim_require_nnan=True,
                nc=nc,
            ))

        devices = jax.devices()[:N_CORES]
        mesh = Mesh(np.asarray(devices), ("core",))
        self._fn = jax.jit(
            shard_map(_body, mesh=mesh,
                      in_specs=(PartitionSpec("core"),) * (n_params + n_outs),
                      out_specs=(PartitionSpec("core"),) * n_outs,
                      check_rep=False),
            donate_argnums=donate, keep_unused=True,
        )

    def run(self, in_maps):
        if not self.axon:
            from concourse import bass_utils
            res = bass_utils.run_bass_kernel_spmd(
                self.nc, in_maps, core_ids=list(range(N_CORES)))
            return res.results
        concat_in = [
            np.concatenate([np.asarray(in_maps[c][n]) for c in range(N_CORES)],
                           axis=0)
            for n in self.in_names
        ] + [np.concatenate([z] * N_CORES, axis=0) for z in self.zero_outs]
        outs = [np.asarray(o) for o in self._fn(*concat_in)]
        per_core = []
        for c in range(N_CORES):
            d = {}
            for name, o in zip(self.out_names, outs):
                rows = o.shape[0] // N_CORES
                d[name] = o[c * rows:(c + 1) * rows]
            per_core.append(d)
        return per_core


def _prep_inputs(x, cos, sin, wq, wk, wv, wo):
    f32 = np.float32
    cosT = np.ascontiguousarray(cos.T.astype(f32))           # [128, S]
    sinm = np.ascontiguousarray(sin.T.astype(f32))
    sinm[0:64] *= -1.0

    wqt = np.ascontiguousarray(wq.T.astype(f32))
    wkt = np.ascontiguousarray(wk.T.astype(f32))
    wvt = np.ascontiguousarray(wv.T.astype(f32))
    wot = np.ascontiguousarray(wo.T.astype(f32))
    ones = np.ones((128, 128), f32)

    xts = [np.ascontiguousarray(x[b].T.astype(f32)) for b in range(B)]

    in_maps = []
    for c in range(N_CORES):
        b, half = c // 2, c % 2
        q0 = half * QLEN
        o0 = QLEN - q0  # start of the "other" half
        xt = np.empty((HID, S), f32)
        xt[:, 0:QLEN] = xts[b][:, q0:q0 + QLEN]
        xt[:, QLEN:S] = xts[b][:, o0:o0 + QLEN]
        ck = np.empty((128, S), f32)
        ck[:, 0:QLEN] = cosT[:, q0:q0 + QLEN]
        ck[:, QLEN:S] = cosT[:, o0:o0 + QLEN]
        sk = np.empty((128, S), f32)
        sk[:, 0:QLEN] = sinm[:, q0:q0 + QLEN]
        sk[:, QLEN:S] = sinm[:, o0:o0 + QLEN]
        in_maps.append({
            "xt": xt, "cosk": ck, "sinkm": sk,
            "wqt": wqt, "wkt": wkt, "wvt": wvt, "wot": wot, "ones": ones,
        })
    return in_maps


def kernel(x, cos, sin, wq, wk, wv, wo):
    if "nc" not in _cache:
        _cache["nc"] = _build()
        _cache["runner"] = _Runner(_cache["nc"])
    runner = _cache["runner"]
    in_maps = _prep_inputs(x, cos, sin, wq, wk, wv, wo)
    results = runner.run(in_maps)
    y = np.empty((B, S, HID), np.float32)
    for c in range(N_CORES):
        b, half = c // 2, c % 2
        q0 = half * QLEN
        y[b, q0:q0 + QLEN, :] = results[c]["y"]
    return y


# revision 21
# speedup vs baseline: 1.1264x; 1.1264x over previous
"""Trainium2 Bass kernel for nn_Attention_55130200211640.

GQA attention block: q/k/v projections + RoPE (theta=1e6) + non-causal
softmax attention (16 q-heads, 4 kv-heads, head_dim 128) + output projection.
B=4, S=2048, HID=2048, fp32 I/O.

Sharding: data-parallel over (batch x 4, query-half x 2) = 8 cores, no
collectives. Each core computes K/V for the full sequence of its batch
(k/v projection duplicated across the pair of cores sharing a batch) and
attention + o_proj for its 1024 queries.

All matmuls run in float32r (TF32-like reduced-precision fp32 path on the
PE: ~1 cycle/row at N>=512 vs 4 for plain fp32, rel. err ~1e-4).

Device dataflow (per core), everything in transposed "contraction-on-
partition" layouts:
  phase 1: X^T resident in SBUF; compute Q^T [hd,i], K^T [d_kv,j] (both
           RoPE'd on DVE using host-transposed cos/sin tables) and
           V [j,d_kv]; bounce all three through DRAM scratch (SBUF is too
           small to hold X^T and the QKV results simultaneously).
  phase 2: per q-head h: S^T[j,i] = K^T_g . Q^T_h on PE; E = exp(S^T/sqrt(D))
           on ACT (scale folded into the activation); U^T[d,i] = sum_j V E
           and Z[i] = sum_j E (ones-matmul) accumulated on PE;
           O^T_h = U^T * recip(Z) on DVE (deferred softmax normalization).
  phase 3: y[i,o] = sum_h O^T_h . wo^T_h accumulated in PSUM over heads.

Token columns are host-permuted so each core's 1024 queries are columns
0:1024 of its X^T — softmax/attention are permutation-invariant over keys,
so K/V in permuted order is fine as long as cos/sin tables are permuted
consistently (they are, per core).
"""

import numpy as np

B, S, HID = 4, 2048, 2048
H, KV, D = 16, 4, 128
REP = H // KV
N_CORES = 8
QLEN = S // 2          # queries per core
CT = HID // 128        # contraction tiles
SCALE = 1.0 / float(np.sqrt(D))

_cache = {}


def _emit(nc, tc, io, scratch=None, skip_cc=False):
    import concourse.mybir as mybir
    import concourse.tile as tile

    F32 = mybir.dt.float32
    F32R = mybir.dt.float32r
    Exp = mybir.ActivationFunctionType.Exp

    xt_d, cosk_d, sinkm_d, wqt_d, wkt_d, wvt_d, wot_d, ones_d, y_d = io

    from contextlib import ExitStack
    ctx = ExitStack()

    # DRAM scratch for the Q/K/V bounce (SBUF cannot hold X^T + outputs).
    dram = ctx.enter_context(tc.tile_pool(name="dram", bufs=1, space="DRAM"))
    if scratch is None:
        scratch = {}
    if "qt_s" not in scratch:
        scratch["qt_s"] = dram.tile([H, 128, QLEN], F32R, name="qt_s")
        scratch["kt_loc"] = dram.tile([KV, 128, QLEN], F32R, name="kt_loc")
        scratch["v_loc"] = dram.tile([QLEN // 128, 128, KV * D], F32R, name="v_loc")
        scratch["kt_g"] = dram.tile([2, KV, 128, QLEN], F32R, name="kt_g")
        scratch["v_g"] = dram.tile([2, QLEN // 128, 128, KV * D], F32R, name="v_g")
    qt_s = scratch["qt_s"]

    const_pool = ctx.enter_context(tc.tile_pool(name="const", bufs=1, side="left"))
    ones_t = const_pool.tile([128, 128], F32R)
    nc.sync.dma_start(ones_t[:], ones_d[:])

    JT = S // 128  # 16 j-tiles

    # ================= phase 1: projections =================
    # Each core projects K/V only for its own 1024 tokens (the query half);
    # the pair of cores sharing a batch exchanges K/V halves with a 2-core
    # AllGather (runs on the collective path, overlapped with the Q
    # projection). X^T for the other half is never touched.
    kt_loc = scratch["kt_loc"]
    v_loc = scratch["v_loc"]
    kt_g = scratch["kt_g"]
    v_g = scratch["v_g"]

    kv_pool = None
    with (
        tc.tile_pool(name="p1xq", bufs=1, side="right") as p1xq,
        tc.tile_pool(name="p1cs", bufs=1, side="right") as p1cs,
        tc.tile_pool(name="p1w", bufs=2, side="right") as p1w,
        tc.tile_pool(name="p1st", bufs=2, side="right") as p1st,
        tc.tile_pool(name="p1ps", bufs=4, space="PSUM", side="right") as p1ps,
    ):
        xt_r = xt_d.rearrange("(ct p) j -> p ct j", p=128)
        wqt_r = wqt_d.rearrange("(ct p) m -> p ct m", p=128)
        wkt_r = wkt_d.rearrange("(ct p) m -> p ct m", p=128)
        wvt_r = wvt_d.rearrange("(ct p) m -> p ct m", p=128)
        k_wts = []
        for g in range(KV):
            wt = p1w.tile([128, CT, 128], F32R, tag="w", bufs=4, name="wt")
            nc.sync.dma_start(wt[:], wkt_r[:, :, g * 128:(g + 1) * 128])
            k_wts.append(wt)
        XQ = p1xq.tile([128, CT, QLEN], F32R)
        nc.sync.dma_start(XQ[:, 0:4, :], xt_r[:, 0:4, :])
        COS = p1cs.tile([128, QLEN], F32)
        nc.sync.dma_start(COS[:], cosk_d[:])
        SINM = p1cs.tile([128, QLEN], F32)
        nc.sync.dma_start(SINM[:], sinkm_d[:])
        for c4 in range(1, 4):
            nc.sync.dma_start(XQ[:, c4 * 4:(c4 + 1) * 4, :],
                              xt_r[:, c4 * 4:(c4 + 1) * 4, :])

        def rope_store(ps, c0, dst):
            """RoPE a [128,512] psum tile ([d, pos], positions c0:c0+512 of
            this core's token half) -> f32r stage -> DMA to scratch."""
            tmp = p1st.tile([128, 512], F32R, tag="tmp", bufs=3)
            stage = p1st.tile([128, 512], F32R, tag="stage", bufs=3)
            nc.vector.tensor_mul(tmp[:], ps[:], COS[:, c0:c0 + 512])
            nc.vector.tensor_mul(stage[0:64, :], ps[64:128, :], SINM[0:64, c0:c0 + 512])
            nc.vector.tensor_mul(stage[64:128, :], ps[0:64, :], SINM[64:128, c0:c0 + 512])
            nc.vector.tensor_add(stage[:], stage[:], tmp[:])
            nc.sync.dma_start(dst, stage[:])

        def k_block(g):
            wt = k_wts[g]
            for jb in range(2):
                j0 = jb * 512
                ps = p1ps.tile([128, 512], F32, tag="p1", name="ps_p1")
                for ct in range(CT):
                    nc.tensor.matmul(
                        ps[:], wt[:, ct, :], XQ[:, ct, j0:j0 + 512],
                        start=(ct == 0), stop=(ct == CT - 1),
                    )
                rope_store(ps, j0, kt_loc[g, :, j0:j0 + 512])

        def v_block(vh):
            d0 = vh * 256
            WVh = p1w.tile([128, CT, 256], F32R, tag="wv", bufs=1, name="wvh")
            nc.scalar.dma_start(WVh[:], wvt_r[:, :, d0:d0 + 256])
            for jl in range(QLEN // 128):
                ps = p1ps.tile([128, 512], F32, tag="p1", name="ps_p1")
                for ct in range(CT):
                    nc.tensor.matmul(
                        ps[:, 0:256], XQ[:, ct, jl * 128:(jl + 1) * 128],
                        WVh[:, ct, :],
                        start=(ct == 0), stop=(ct == CT - 1),
                    )
                stage = p1st.tile([128, 256], F32R, tag="vstage", bufs=3)
                nc.scalar.copy(stage[:], ps[:, 0:256])
                nc.scalar.dma_start(v_loc[jl, :, d0:d0 + 256], stage[:])

        if not skip_cc:
            k_block(0)
            k_block(1)
            v_block(0)
            k_block(2)
            k_block(3)
            v_block(1)

            # Exchange K/V halves with the pair core.
            nc.gpsimd.collective_compute(
                "AllGather", mybir.AluOpType.bypass,
                replica_groups=[[0, 1], [2, 3], [4, 5], [6, 7]],
                ins=[kt_loc[:]], outs=[kt_g[:]],
            )
            nc.gpsimd.collective_compute(
                "AllGather", mybir.AluOpType.bypass,
                replica_groups=[[0, 1], [2, 3], [4, 5], [6, 7]],
                ins=[v_loc[:]], outs=[v_g[:]],
            )

        kv_pool = ctx.enter_context(tc.tile_pool(name="kv", bufs=1, side="left"))
        KT = kv_pool.tile([128, KV, S], F32R)
        VV = kv_pool.tile([128, JT, KV * D], F32R)

        # --- Q^T (K/V halves stream in from the gather meanwhile) ---
        for h in range(H):
            wt = p1w.tile([128, CT, 128], F32R, tag="w", bufs=4, name="wt")
            nc.sync.dma_start(wt[:], wqt_r[:, :, h * 128:(h + 1) * 128])
            if 1 <= h <= 8:
                half, g = (h - 1) // 4, (h - 1) % 4
                nc.sync.dma_start(KT[:, g, half * QLEN:(half + 1) * QLEN],
                                  kt_g[half, g, :, :])
            elif 9 <= h <= 12:
                for jj in range(4):
                    jt = (h - 9) * 4 + jj
                    nc.sync.dma_start(VV[:, jt, :], v_g[jt // 8, jt % 8, :, :])
            for ib in range(2):
                i0 = ib * 512
                ps = p1ps.tile([128, 512], F32, tag="p1", name="ps_p1")
                for ct in range(CT):
                    nc.tensor.matmul(
                        ps[:], wt[:, ct, :], XQ[:, ct, i0:i0 + 512],
                        start=(ct == 0), stop=(ct == CT - 1),
                    )
                rope_store(ps, i0, qt_s[h, :, i0:i0 + 512])

    # ================= phase 2: attention =================
    o_pool = ctx.enter_context(tc.tile_pool(name="ot", bufs=1, side="left"))
    OT = o_pool.tile([128, H, QLEN], F32R)

    with (
        tc.tile_pool(name="p2q", bufs=3, side="right") as p2q,
        tc.tile_pool(name="p2e", bufs=8, side="right") as p2e,
        tc.tile_pool(name="p2r", bufs=2, side="right") as p2r,
        tc.tile_pool(name="p2ps_s", bufs=2, space="PSUM", side="left") as p2ps_s,
        tc.tile_pool(name="p2ps_uz", bufs=1, space="PSUM", side="right") as p2ps_uz,
    ):
        for h in range(H):
            g = h // REP
            QH = p2q.tile([128, QLEN], F32R, tag="qh", bufs=2)
            nc.sync.dma_start(QH[:], qt_s[h, :, :])
            U_ps = p2ps_uz.tile([128, QLEN], F32, tag="U", bufs=1, name="ps_U")
            Z_ps = p2ps_uz.tile([128, QLEN], F32, tag="Z", bufs=1, name="ps_Z")
            Es = []
            for jt in range(JT):
                S_ps = p2ps_s.tile([128, QLEN], F32, tag="S", bufs=2, name="ps_S")
                kt_sl = KT[:, g, jt * 128:(jt + 1) * 128]
                nc.tensor.matmul(S_ps[:, 0:512], kt_sl, QH[:, 0:512],
                                 start=True, stop=True)
                nc.tensor.matmul(S_ps[:, 512:1024], kt_sl, QH[:, 512:1024],
                                 start=True, stop=True)
                E = p2e.tile([128, QLEN], F32R, tag="e", bufs=16)
                nc.scalar.activation(E[:], S_ps[:], Exp, scale=SCALE)
                Es.append(E)
                v_sl = VV[:, jt, g * 128:(g + 1) * 128]
                st, sp = (jt == 0), (jt == JT - 1)
                nc.tensor.matmul(U_ps[:, 0:512], v_sl, E[:, 0:512],
                                 start=st, stop=sp)
                nc.tensor.matmul(U_ps[:, 512:1024], v_sl, E[:, 512:1024],
                                 start=st, stop=sp)
            # Z burst: runs dense on PE, filling the next head's ACT-paced
            # stream (the scheduler interleaves them across heads).
            for jt in range(JT):
                E = Es[jt]
                st, sp = (jt == 0), (jt == JT - 1)
                nc.tensor.matmul(Z_ps[:, 0:512], ones_t[:], E[:, 0:512],
                                 start=st, stop=sp)
                nc.tensor.matmul(Z_ps[:, 512:1024], ones_t[:], E[:, 512:1024],
                                 start=st, stop=sp)
            RZ = p2r.tile([128, QLEN], F32, tag="rz", bufs=1)
            nc.vector.reciprocal_approx_fast(RZ[:], Z_ps[:])
            nc.vector.tensor_mul(OT[:, h, :], U_ps[:], RZ[:])

    # ================= phase 3: output projection =================
    with (
        tc.tile_pool(name="p3w", bufs=18, side="right") as p3w,
        tc.tile_pool(name="p3y", bufs=8, side="right") as p3y,
        tc.tile_pool(name="p3ps", bufs=8, space="PSUM", side="right") as p3ps,
    ):
        for ob in range(4):
            o0 = ob * 512
            Ws = []
            for h in range(H):
                W = p3w.tile([128, 512], F32R, tag="wo", bufs=18, name="wo_t")
                nc.sync.dma_start(W[:], wot_d[h * 128:(h + 1) * 128, o0:o0 + 512])
                Ws.append(W)
            for half in range(2):
                pss = [p3ps.tile([128, 512], F32, tag="y", bufs=8, name="ps_y")
                       for _ in range(4)]
                for h in range(H):
                    for q in range(4):
                        it = half * 4 + q
                        nc.tensor.matmul(
                            pss[q][:], OT[:, h, it * 128:(it + 1) * 128],
                            Ws[h][:],
                            start=(h == 0), stop=(h == H - 1),
                        )
                for q in range(4):
                    it = half * 4 + q
                    yt = p3y.tile([128, 512], F32, tag="yt", bufs=8)
                    if q % 2 == 0:
                        nc.vector.tensor_copy(yt[:], pss[q][:])
                    else:
                        nc.scalar.copy(yt[:], pss[q][:])
                    nc.scalar.dma_start(
                        y_d[it * 128:(it + 1) * 128, o0:o0 + 512], yt[:])

    ctx.close()


def _build(repeat=1):
    import concourse.mybir as mybir
    import concourse.tile as tile
    from concourse import bacc

    F32 = mybir.dt.float32
    F32R = mybir.dt.float32r

    nc = bacc.Bacc("TRN2", target_bir_lowering=False, debug=False, num_devices=N_CORES)
    xt_d = nc.dram_tensor("xt", [HID, QLEN], F32R, kind="ExternalInput").ap()
    cosk_d = nc.dram_tensor("cosk", [128, QLEN], F32, kind="ExternalInput").ap()
    sinkm_d = nc.dram_tensor("sinkm", [128, QLEN], F32, kind="ExternalInput").ap()
    wqt_d = nc.dram_tensor("wqt", [HID, H * D], F32R, kind="ExternalInput").ap()
    wkt_d = nc.dram_tensor("wkt", [HID, KV * D], F32R, kind="ExternalInput").ap()
    wvt_d = nc.dram_tensor("wvt", [HID, KV * D], F32R, kind="ExternalInput").ap()
    wot_d = nc.dram_tensor("wot", [H * D, HID], F32R, kind="ExternalInput").ap()
    ones_d = nc.dram_tensor("ones", [128, 128], F32R, kind="ExternalInput").ap()
    y_d = nc.dram_tensor("y", [QLEN, HID], F32, kind="ExternalOutput").ap()

    with tile.TileContext(nc) as tc:
        scratch = {}
        for r in range(repeat):
            _emit(nc, tc, (xt_d, cosk_d, sinkm_d, wqt_d, wkt_d, wvt_d, wot_d,
                           ones_d, y_d), scratch=scratch, skip_cc=(r > 0))
    nc.compile()
    return nc


class _Runner:
    """Persistent-jit PJRT executor (axon) / NRT executor (native)."""

    def __init__(self, nc):
        self.nc = nc
        from concourse._compat import axon_active
        self.axon = axon_active()
        if not self.axon:
            return
        import jax
        from jax.sharding import Mesh, PartitionSpec
        from jax.experimental.shard_map import shard_map
        import concourse.mybir as mybir
        from concourse.bass2jax import (
            _bass_exec_p, install_neuronx_cc_hook, partition_id_tensor)

        install_neuronx_cc_hook()
        partition_name = (nc.partition_id_tensor.name
                          if nc.partition_id_tensor else None)
        in_names, out_names, out_avals, zero_outs = [], [], [], []
        for alloc in nc.m.functions[0].allocations:
            if not isinstance(alloc, mybir.MemoryLocationSet):
                continue
            name = alloc.memorylocations[0].name
            if alloc.kind == "ExternalInput":
                if name != partition_name:
                    in_names.append(name)
            elif alloc.kind == "ExternalOutput":
                shape = tuple(alloc.tensor_shape)
                dtype = mybir.dt.np(alloc.dtype)
                out_names.append(name)
                out_avals.append(jax.core.ShapedArray(shape, dtype))
                zero_outs.append(np.zeros(shape, dtype))
        self.in_names, self.out_names = in_names, out_names
        self.zero_outs = zero_outs
        n_params, n_outs = len(in_names), len(out_names)
        all_in = in_names + out_names
        if partition_name is not None:
            all_in.append(partition_name)
        donate = tuple(range(n_params, n_params + n_outs))

        def _body(*args):
            operands = list(args)
            if partition_name is not None:
                operands.append(partition_id_tensor())
            return tuple(_bass_exec_p.bind(
                *operands,
                out_avals=tuple(out_avals),
                in_names=tuple(all_in),
                out_names=tuple(out_names),
                lowering_input_output_aliases=(),
                sim_require_finite=True,
                sim_require_nnan=True,
                nc=nc,
            ))

        devices = jax.devices()[:N_CORES]
        mesh = Mesh(np.asarray(devices), ("core",))
        self._fn = jax.jit(
            shard_map(_body, mesh=mesh,
                      in_specs=(PartitionSpec("core"),) * (n_params + n_outs),
                      out_specs=(PartitionSpec("core"),) * n_outs,
                      check_rep=False),
            donate_argnums=donate, keep_unused=True,
        )

    def run(self, in_maps):
        if not self.axon:
            from concourse import bass_utils
            res = bass_utils.run_bass_kernel_spmd(
                self.nc, in_maps, core_ids=list(range(N_CORES)))
            return res.results
        concat_in = [
            np.concatenate([np.asarray(in_maps[c][n]) for c in range(N_CORES)],
                           axis=0)
            for n in self.in_names
        ] + [np.concatenate([z] * N_CORES, axis=0) for z in self.zero_outs]
        outs = [np.asarray(o) for o in self._fn(*concat_in)]
        per_core = []
        for c in range(N_CORES):
            d = {}
            for name, o in zip(self.out_names, outs):
                rows = o.shape[0] // N_CORES
                d[name] = o[c * rows:(c + 1) * rows]
            per_core.append(d)
        return per_core


def _prep_inputs(x, cos, sin, wq, wk, wv, wo):
    f32 = np.float32
    cosT = np.ascontiguousarray(cos.T.astype(f32))           # [128, S]
    sinm = np.ascontiguousarray(sin.T.astype(f32))
    sinm[0:64] *= -1.0

    wqt = np.ascontiguousarray(wq.T.astype(f32))
    wkt = np.ascontiguousarray(wk.T.astype(f32))
    wvt = np.ascontiguousarray(wv.T.astype(f32))
    wot = np.ascontiguousarray(wo.T.astype(f32))
    ones = np.ones((128, 128), f32)

    in_maps = []
    for c in range(N_CORES):
        b, half = c // 2, c % 2
        q0 = half * QLEN
        xt = np.ascontiguousarray(x[b].T[:, q0:q0 + QLEN].astype(f32))
        ck = np.ascontiguousarray(cosT[:, q0:q0 + QLEN])
        sk = np.ascontiguousarray(sinm[:, q0:q0 + QLEN])
        in_maps.append({
            "xt": xt, "cosk": ck, "sinkm": sk,
            "wqt": wqt, "wkt": wkt, "wvt": wvt, "wot": wot, "ones": ones,
        })
    return in_maps


def kernel(x, cos, sin, wq, wk, wv, wo):
    if "nc" not in _cache:
        _cache["nc"] = _build()
        _cache["runner"] = _Runner(_cache["nc"])
    runner = _cache["runner"]
    in_maps = _prep_inputs(x, cos, sin, wq, wk, wv, wo)
    results = runner.run(in_maps)
    y = np.empty((B, S, HID), np.float32)
    for c in range(N_CORES):
        b, half = c // 2, c % 2
        q0 = half * QLEN
        y[b, q0:q0 + QLEN, :] = results[c]["y"]
    return y
